# revision 3
# baseline (speedup 1.0000x reference)
"""BatchedKiloNeRF Trainium2 kernel.

Strategy (expert-parallel, host routing, bf16, block-diagonal quads):
  - 4096 tiny MLPs ("experts"), 131072 points routed by model_indices.
  - PE cost is dominated by the 128-col LDWEIGHTS (~53ns with FWL), so
    experts are packed 4-per-matmul: quad q stacks experts b=0..3 on
    partition bands 32b with a block-diagonal stationary [128, 128];
    one matmul streams C point-columns for 4 experts at once.
  - Host sorts experts by point count and packs groups of EPG=32 (8 quads)
    per core; points padded to the group capacity C (max count in the
    8*EPG-expert window). Hidden states are [128, 8C] bf16 SBUF tiles:
    partition band 32b = expert band, C-column segment q = quad.
  - PSUM: each step claims one full bank; 8-bank global rotation. All
    matmuls of a step are closed single-shots except the view step:
    ident (start) and viewA (stop) are emitted adjacently per quad
    because start=True clears has_written for the WHOLE bank - any
    interleaving of other start=True matmuls between an open pair
    loses the first contribution.
  - Weights ship dense from host: bdlv packs the L1 and viewA block-
    diagonal stationaries column-adjacent per group ([128, 2048]/group)
    so chunked DMAs have 16KB per-partition runs. L0 (K=16) and
    sigma/rgb (narrow M) blobs ship as separate small tensors.
  - vparts: the view-direction partial (Wv[:,32:] @ views + folded bias)
    is host-computed per point and injected into the view PSUM bank via
    an identity-stationary matmul (keeps the PE on uniform K=128 and
    dodges the has_written hazard).
  - DMA: only the two HWDGE queues (sync=SP, scalar=ACT) trigger DMAs;
    gpsimd SWDGE measured ~88GB/s with ~1us engine time per DMA - not
    used. Big tensors alternate between the queues; triggers are
    priority-ordered by pipeline window. Output is bf16 and streamed
    per window instead of one tail DMA.
  - Engine split: vector does the big PSUM->SBUF relu/copies (c_l0,
    c_l1, c_view), scalar does the narrow sigma/rgb copies + DMA
    triggers. gpsimd has no PSUM port and idles.
  - Biases: L0 bias rides in the matmul via a constant-1 input row;
    feat layer is folded into the view layer on the host. L1 bias is
    zero in practice (fast path: single relu copy); nonzero b1 falls
    back to per-quad tensor_scalar ops. sigma/rgb biases are added on
    host.
"""

import sys

import numpy as np
import ml_dtypes

BF16 = ml_dtypes.bfloat16

for _p in ("/opt/trn_rl_repo",):
    if _p not in sys.path:
        sys.path.append(_p)

NUM_MODELS = 4096
W = 32
N = 131072
NCORES = 8
EPG = 32               # experts per group per core (8 quads)
QPG = EPG // 4         # quads per group
NGROUPS = 512 // EPG
WIN = NCORES * EPG     # experts per capacity window

# wblob per group [128, WBLOB_F] bf16 (dense, host-built):
#   sigma lhsT [0:4Q)     rows 32b+h, col 4q+b
#   rgb lhsT   [4Q:16Q)   rows 32b+h, col 12q+3b+r
#   b1 bias    [16Q:17Q)  rows 32b+h, col q
WBLOB_F = 17 * QPG
# sblob per group [16, 128Q]: w0aug lhsT, rows 4b+k, col 128q+32b+h
SBLOB_F = 128 * QPG
# bdlv per group [128, 2*128*QPG]: L1 block-diag | viewA block-diag
BDG = 128 * QPG
BDLV_F = 2 * BDG
BANK = 512
PIPE = 4


def _prep(x, model_indices, pts_w0, pts_b0, pts_w1, pts_b1,
          feat_w, feat_b, sigma_w, sigma_b, view_w, view_b, rgb_w, rgb_b):
    """Host-side routing + packing. Returns per-core device arrays and
    decode info."""
    x = np.asarray(x, np.float32)
    idx = np.asarray(model_indices).astype(np.int64)
    counts = np.bincount(idx, minlength=NUM_MODELS)

    expert_order = np.argsort(-counts, kind="stable")  # descending count
    caps = np.empty(NGROUPS, np.int64)
    for k in range(NGROUPS):
        win = expert_order[WIN * k:WIN * (k + 1)]
        c = int(counts[win].max())
        caps[k] = max(4, -(-c // 4) * 4)  # round up to multiple of 4, >=4
    assert caps.max() * QPG <= BANK, "group capacity exceeds one PSUM bank"
    colstart = np.concatenate([[0], np.cumsum(QPG * caps)])
    w_tot = int(colstart[-1])

    order_pts = np.argsort(idx, kind="stable")
    starts = np.concatenate([[0], np.cumsum(counts)])

    # fold the feat layer into the view layer on the host:
    #   view(h) = relu(Wv [feat(h); views] + bv)
    #           = relu((Wv[:, :32] @ Wf) h + WvB views + (bv + Wv[:, :32] bf))
    vb_fold = view_b + np.einsum("goh,gh->go", view_w[:, :, :W], feat_b)
    vwA_fold = np.einsum("gox,gxh->goh", view_w[:, :, :W], feat_w)
    # host-computed view-direction partial per point:
    #   vpart = Wv[:, 32:] @ views + vb_fold   (injected into PSUM on device
    #   via an identity-stationary matmul, so both view matmuls are K=128)
    vw_g = view_w[idx][:, :, W:]                      # [N, 32, 3]
    vpart_all = (np.einsum("nij,nj->ni", vw_g, x[:, 3:6])
                 + vb_fold[idx]).astype(np.float32)   # [N, 32]
    w0aug = np.concatenate(
        [np.transpose(pts_w0, (0, 2, 1)), pts_b0[:, None, :]], axis=1
    ).astype(np.float32)                      # [E, 4, 32] lhsT rows: xyz+bias
    w1T = np.transpose(pts_w1, (0, 2, 1)).astype(np.float32)    # [E,32,32]
    vwAT = np.transpose(vwA_fold, (0, 2, 1)).astype(np.float32)
    sigT = np.transpose(sigma_w, (0, 2, 1)).astype(np.float32)  # [E,32,1]
    rgbT = np.transpose(rgb_w, (0, 2, 1)).astype(np.float32)    # [E,32,3]
    b1 = np.asarray(pts_b1, np.float32)

    per_core = []
    decode = []
    for c in range(NCORES):
        gq = np.stack([expert_order[WIN * k + EPG * c: WIN * k + EPG * (c + 1)]
                       for k in range(NGROUPS)])  # [NGROUPS, EPG]

        wblob = np.zeros((NGROUPS, 128, WBLOB_F), np.float32)
        sblob = np.zeros((NGROUPS, 16, SBLOB_F), np.float32)
        bdlv = np.zeros((NGROUPS, 128, BDLV_F), np.float32)
        xpts = np.zeros((16, w_tot), np.float32)
        vparts = np.zeros((128, w_tot), np.float32)
        xpts[3::4, :] = 1.0   # constant-1 rows for bias-in-matmul
        for k in range(NGROUPS):
            C = int(caps[k])
            col = int(colstart[k])
            for l in range(EPG):
                gid = int(gq[k, l])
                q, b = l // 4, l % 4
                wblob[k, 32 * b:32 * b + 32, 4 * q + b] = sigT[gid, :, 0]
                wblob[k, 32 * b:32 * b + 32, 4 * QPG + 12 * q + 3 * b:
                      4 * QPG + 12 * q + 3 * b + 3] = rgbT[gid]
                wblob[k, 32 * b:32 * b + 32, 16 * QPG + q] = b1[gid]
                sblob[k, 4 * b:4 * b + 4, 128 * q + 32 * b:
                      128 * q + 32 * b + 32] = w0aug[gid]
                bdlv[k, 32 * b:32 * b + 32,
                     128 * q + 32 * b:128 * q + 32 * b + 32] = w1T[gid]
                bdlv[k, 32 * b:32 * b + 32,
                     BDG + 128 * q + 32 * b:BDG + 128 * q + 32 * b + 32] = \
                    vwAT[gid]
                cnt = int(counts[gid])
                pts = order_pts[starts[gid]:starts[gid] + cnt]
                cq = col + q * C
                if cnt:
                    xpts[4 * b:4 * b + 3, cq:cq + cnt] = x[pts, :3].T
                    vparts[32 * b:32 * b + 32, cq:cq + cnt] = vpart_all[pts].T
                decode.append((c, gid, pts, q, b, cq, cnt))
        per_core.append(dict(
            xpts=xpts.astype(BF16), vparts=vparts.astype(BF16),
            ident=np.eye(128, dtype=np.float32).astype(BF16),
            wblob=wblob.transpose(1, 0, 2).reshape(128, NGROUPS * WBLOB_F)
                       .astype(BF16),
            sblob=sblob.transpose(1, 0, 2).reshape(16, NGROUPS * SBLOB_F)
                       .astype(BF16),
            bdlv=bdlv.transpose(1, 0, 2)
                     .reshape(128, NGROUPS * BDLV_F).astype(BF16)))

    b1_zero = not np.any(b1)
    return per_core, decode, caps, colstart, w_tot, b1_zero


def _build_nc(caps, w_tot, b1_zero):
    import concourse.mybir as mybir
    import concourse.tile as tile
    from concourse import bacc
    from contextlib import ExitStack

    f32 = mybir.dt.float32
    bf16 = mybir.dt.bfloat16
    ADD = mybir.AluOpType.add
    MAX = mybir.AluOpType.max

    nc = bacc.Bacc("TRN2", target_bir_lowering=False)
    xpts_d = nc.declare_dram_parameter("xpts", [16, w_tot], bf16, isOutput=False)
    vparts_d = nc.declare_dram_parameter("vparts", [128, w_tot], bf16,
                                         isOutput=False)
    ident_d = nc.declare_dram_parameter("ident", [128, 128], bf16,
                                        isOutput=False)
    wblob_d = nc.declare_dram_parameter("wblob", [128, NGROUPS * WBLOB_F], bf16,
                                        isOutput=False)
    sblob_d = nc.declare_dram_parameter("sblob", [16, NGROUPS * SBLOB_F], bf16,
                                        isOutput=False)
    bdlv_d = nc.declare_dram_parameter("bdlv", [128, NGROUPS * BDLV_F], bf16,
                                       isOutput=False)
    out_d = nc.declare_dram_parameter("out", [16, w_tot], bf16, isOutput=True)

    with tile.TileContext(nc) as tc, ExitStack() as ctx:
        const = ctx.enter_context(tc.tile_pool(name="const", bufs=1))
        hpool = ctx.enter_context(tc.tile_pool(name="h", bufs=4))
        pspool = ctx.enter_context(tc.tile_pool(name="ps", bufs=1, space="PSUM"))
        psall = pspool.tile([128, 8 * BANK], f32, tag="psall")
        # Global bank rotation; every step copies its slot out within the
        # step, so slot lifetime is one wave and 8 banks cover PIPE=4
        # windows without collisions.
        step_ctr = [0]

        def slot():
            bank = step_ctr[0] % 8
            step_ctr[0] += 1

            def mm_out(part_lo, m, q, C):
                base = bank * BANK + q * C
                return psall[part_lo:part_lo + m, base:base + C]

            def copy_src(part_lo, m, width):
                return psall[part_lo:part_lo + m,
                             bank * BANK:bank * BANK + width]

            return mm_out, copy_src

        xt = const.tile([16, w_tot], bf16)
        vpt = const.tile([128, w_tot], bf16)
        ident = const.tile([128, 128], bf16)
        wt_all = const.tile([128, NGROUPS * WBLOB_F], bf16)
        st_all = const.tile([16, NGROUPS * SBLOB_F], bf16)
        bdlv = const.tile([128, NGROUPS * BDLV_F], bf16)
        otr = const.tile([12, w_tot], bf16)
        ots = const.tile([4, w_tot], bf16)
        colstarts = np.concatenate([[0], np.cumsum(QPG * np.asarray(caps))])

        # ── Priority-ordered input DMAs on the two HWDGE queues ──────────
        # (gpsimd SWDGE measured ~88GB/s + ~1us engine time per DMA; the
        # vector engine has no DGE on TRN2.)  Big bdlv chunks alternate
        # between the queues so both carry ~half the bytes; within each
        # queue triggers are ordered by pipeline window.
        small_r = [(0, 1), (1, 4), (4, 10), (10, 16)]
        vp_r = [(0, 1), (1, 2), (2, 4), (4, 8), (8, 12), (12, 16)]
        bd_r = [(0, 1), (1, 2), (2, 3), (3, 4), (4, 6), (6, 8),
                (8, 10), (10, 12), (12, 14), (14, 16)]

        def win_of(rng):
            return rng[0]

        dmas = []  # (window_priority, order, queue, dst, src)
        dmas.append((0, 0, 0, ident[:], ident_d[:]))
        for i, (a, b) in enumerate(small_r):
            xlo, xhi = colstarts[a], colstarts[b]
            dmas.append((a, 1, 0, xt[:, xlo:xhi], xpts_d[:, xlo:xhi]))
            dmas.append((a, 2, 0, st_all[:, a * SBLOB_F:b * SBLOB_F],
                         sblob_d[:, a * SBLOB_F:b * SBLOB_F]))
            dmas.append((a, 4, 1, wt_all[:, a * WBLOB_F:b * WBLOB_F],
                         wblob_d[:, a * WBLOB_F:b * WBLOB_F]))
        for i, (a, b) in enumerate(bd_r):
            dmas.append((a, 3, i % 2, bdlv[:, a * BDLV_F:b * BDLV_F],
                         bdlv_d[:, a * BDLV_F:b * BDLV_F]))
        for i, (a, b) in enumerate(vp_r):
            xlo, xhi = colstarts[a], colstarts[b]
            dmas.append((a, 5, (i + 1) % 2,
                         vpt[:, xlo:xhi], vparts_d[:, xlo:xhi]))
        dmas.sort(key=lambda t: (t[0], t[1]))
        engines = [nc.sync, nc.scalar]
        for (_, _, q, dst, src) in dmas:
            engines[q].dma_start(out=dst, in_=src)

        def group_steps(g):
            C = int(caps[g])
            WC = QPG * C
            col = int(colstarts[g])
            wt = wt_all[:, g * WBLOB_F:(g + 1) * WBLOB_F]
            st = st_all[:, g * SBLOB_F:(g + 1) * SBLOB_F]
            l1w = bdlv[:, g * BDLV_F:g * BDLV_F + BDG]
            vaw = bdlv[:, g * BDLV_F + BDG:(g + 1) * BDLV_F]
            state = {}

            def s_l0():
                mm0, cp0 = slot()
                for q in range(QPG):
                    nc.tensor.matmul(
                        out=mm0(0, 128, q, C),
                        lhsT=st[0:16, 128 * q:128 * q + 128],
                        rhs=xt[0:16, col + q * C:col + q * C + C],
                        start=True, stop=True, skip_group_check=True,
                        tile_position=(0, 0))
                state["cp0"] = cp0

            def c_l0():
                h1 = hpool.tile([128, WC], bf16, tag="h1")
                nc.vector.tensor_scalar_max(h1[:], state.pop("cp0")(0, 128, WC),
                                            0.0)
                state["h1"] = h1

            def s_l1():
                h1 = state.pop("h1")
                mm1, cp1 = slot()
                for q in range(QPG):
                    nc.tensor.matmul(
                        out=mm1(0, 128, q, C),
                        lhsT=l1w[:, 128 * q:128 * q + 128],
                        rhs=h1[:, q * C:q * C + C],
                        start=True, stop=True, skip_group_check=True,
                        tile_position=(0, 0))
                state["cp1"] = cp1

            def c_l1():
                cp1 = state.pop("cp1")
                h2 = hpool.tile([128, WC], bf16, tag="h2")
                if b1_zero:
                    nc.vector.tensor_scalar_max(h2[:], cp1(0, 128, WC), 0.0)
                else:
                    for q in range(QPG):
                        nc.vector.tensor_scalar(
                            out=h2[:, q * C:q * C + C],
                            in0=cp1(0, 128, WC)[:, q * C:q * C + C],
                            scalar1=wt[:, 16 * QPG + q:16 * QPG + q + 1],
                            scalar2=0.0, op0=ADD, op1=MAX)
                state["h2"] = h2

            def s_sigma():
                h2 = state["h2"]
                mms_, cps = slot()
                for q in range(QPG):
                    nc.tensor.matmul(
                        out=mms_(0, 4, q, C),
                        lhsT=wt[:, 4 * q:4 * q + 4],
                        rhs=h2[:, q * C:q * C + C],
                        start=True, stop=True, skip_group_check=True,
                        tile_position=(0, 0))
                state["cps"] = cps

            def c_sigma():
                nc.scalar.copy(ots[:, col:col + WC],
                               state.pop("cps")(0, 4, WC))

            def s_view():
                h2 = state.pop("h2")
                mmv, cpv = slot()
                # ident (start) and viewA (stop) stay adjacent per quad:
                # start=True clears has_written for the whole bank, so any
                # other start=True between an open pair loses the ident
                # contribution. Both are K=128 (uniform row size keeps the
                # PE pipeline from flushing between matmuls).
                for q in range(QPG):
                    nc.tensor.matmul(
                        out=mmv(0, 128, q, C),
                        lhsT=ident[:],
                        rhs=vpt[:, col + q * C:col + q * C + C],
                        start=True, stop=False, skip_group_check=True,
                        tile_position=(0, 0))
                    nc.tensor.matmul(
                        out=mmv(0, 128, q, C),
                        lhsT=vaw[:, 128 * q:128 * q + 128],
                        rhs=h2[:, q * C:q * C + C],
                        start=False, stop=True, skip_group_check=True,
                        tile_position=(0, 0))
                state["cpv"] = cpv

            def c_view():
                hv = hpool.tile([128, WC], bf16, tag="hv")
                nc.vector.tensor_scalar_max(hv[:], state.pop("cpv")(0, 128, WC),
                                            0.0)
                state["hv"] = hv

            def s_rgb():
                hv = state.pop("hv")
                mmr, cpr = slot()
                for q in range(QPG):
                    nc.tensor.matmul(
                        out=mmr(0, 12, q, C),
                        lhsT=wt[:, 4 * QPG + 12 * q:4 * QPG + 12 * q + 12],
                        rhs=hv[:, q * C:q * C + C],
                        start=True, stop=True, skip_group_check=True,
                        tile_position=(0, 0))
                state["cpr"] = cpr

            def c_rgb():
                nc.scalar.copy(otr[:, col:col + WC],
                               state.pop("cpr")(0, 12, WC))

            return [(s_l0, c_l0), (s_l1, c_l1), (s_sigma, c_sigma),
                    (s_view, c_view), (s_rgb, c_rgb)]

        for base in range(0, NGROUPS, PIPE):
            top = min(base + PIPE, NGROUPS)
            window = [group_steps(g) for g in range(base, top)]
            for stepi in range(5):
                for steps in window:
                    steps[stepi][0]()   # matmuls of the wave first
                for steps in window:
                    steps[stepi][1]()   # then the copies (waits satisfied)
            # stream this window's output while later windows compute
            lo, hi = colstarts[base], colstarts[top]
            nc.sync.dma_start(out=out_d[0:12, lo:hi], in_=otr[:, lo:hi])
            nc.sync.dma_start(out=out_d[12:16, lo:hi], in_=ots[:, lo:hi])

    nc.compile()
    return nc


def _decode_out(results, decode, sigma_b, rgb_b):
    y = np.empty((N, 4), np.float32)
    outs = [np.asarray(r["out"], dtype=np.float32) for r in results]
    for (c, gid, pts, q, b, cq, cnt) in decode:
        if cnt == 0:
            continue
        o = outs[c]
        y[pts, 0:3] = o[3 * b:3 * b + 3, cq:cq + cnt].T + rgb_b[gid]
        y[pts, 3] = o[12 + b, cq:cq + cnt] + sigma_b[gid, 0]
    return y


def kernel(**inputs):
    from concourse.bass_utils import run_bass_kernel_spmd

    per_core, decode, caps, colstart, w_tot, b1_zero = _prep(**inputs)
    nc = _build_nc(caps, w_tot, b1_zero)
    in_maps = [per_core[c] for c in range(NCORES)]
    res = run_bass_kernel_spmd(nc, in_maps, list(range(NCORES)))
    return _decode_out(res.results, decode,
                       np.asarray(inputs["sigma_b"], np.float32),
                       np.asarray(inputs["rgb_b"], np.float32))


# ---------------------------------------------------------------------------
# numpy emulation of the device program (for layout validation in test.py)
def _emulate_core(arrs, caps, w_tot):
    arrs = {k: np.asarray(v, np.float32) for k, v in arrs.items()}
    xt = arrs["xpts"]
    vpt = arrs["vparts"]
    bdlv = arrs["bdlv"]
    out = np.zeros((16, w_tot), np.float32)
    col = 0
    for g in range(NGROUPS):
        C = int(caps[g])
        WC = QPG * C
        wt = arrs["wblob"][:, g * WBLOB_F:(g + 1) * WBLOB_F]
        st = arrs["sblob"][:, g * SBLOB_F:(g + 1) * SBLOB_F]
        l1w = bdlv[:, g * BDLV_F:g * BDLV_F + BDG]
        vaw = bdlv[:, g * BDLV_F + BDG:(g + 1) * BDLV_F]

        h1 = np.zeros((128, WC), np.float32)
        for q in range(QPG):
            h1[:, q * C:q * C + C] = (
                st[:, 128 * q:128 * q + 128].T
                @ xt[:, col + q * C:col + q * C + C])
        h1 = np.maximum(h1, 0).astype(BF16).astype(np.float32)
        h2 = np.zeros((128, WC), np.float32)
        for q in range(QPG):
            h2[:, q * C:q * C + C] = (
                l1w[:, 128 * q:128 * q + 128].T @ h1[:, q * C:q * C + C]
                + wt[:, 16 * QPG + q:16 * QPG + q + 1])
        h2 = np.maximum(h2, 0).astype(BF16).astype(np.float32)
        for q in range(QPG):
            out[12:16, col + q * C:col + q * C + C] = (
                wt[:, 4 * q:4 * q + 4].T @ h2[:, q * C:q * C + C])
        hv = np.zeros((128, WC), np.float32)
        for q in range(QPG):
            hv[:, q * C:q * C + C] = (
                vaw[:, 128 * q:128 * q + 128].T @ h2[:, q * C:q * C + C]
                + vpt[:, col + q * C:col + q * C + C])
        hv = np.maximum(hv, 0).astype(BF16).astype(np.float32)
        for q in range(QPG):
            out[0:12, col + q * C:col + q * C + C] = (
                wt[:, 4 * QPG + 12 * q:4 * QPG + 12 * q + 12].T
                @ hv[:, q * C:q * C + C])
        col += WC
    return out.astype(BF16)


def kernel_emulated(**inputs):
    per_core, decode, caps, colstart, w_tot, b1_zero = _prep(**inputs)
    results = [{"out": _emulate_core(per_core[c], caps, w_tot)}
               for c in range(NCORES)]
    return _decode_out(results, decode,
                       np.asarray(inputs["sigma_b"], np.float32),
                       np.asarray(inputs["rgb_b"], np.float32))


# revision 4
# speedup vs baseline: 1.0137x; 1.0137x over previous
"""BatchedKiloNeRF Trainium2 kernel.

Strategy (expert-parallel, host routing, bf16, block-diagonal quads):
  - 4096 tiny MLPs ("experts"), 131072 points routed by model_indices.
  - PE cost is dominated by the 128-col LDWEIGHTS (~53ns with FWL), so
    experts are packed 4-per-matmul: quad q stacks experts b=0..3 on
    partition bands 32b with a block-diagonal stationary [128, 128];
    one matmul streams C point-columns for 4 experts at once.
  - Host sorts experts by point count and packs groups of EPG=32 (8 quads)
    per core; points padded to the group capacity C (max count in the
    8*EPG-expert window). Hidden states are [128, 8C] bf16 SBUF tiles:
    partition band 32b = expert band, C-column segment q = quad.
  - PSUM: each step claims one full bank; 8-bank global rotation. All
    matmuls of a step are closed single-shots except the view step:
    ident (start) and viewA (stop) are emitted adjacently per quad
    because start=True clears has_written for the WHOLE bank - any
    interleaving of other start=True matmuls between an open pair
    loses the first contribution.
  - Weights ship dense from host: bdlv packs the L1 and viewA block-
    diagonal stationaries column-adjacent per group ([128, 2048]/group)
    so chunked DMAs have 16KB per-partition runs. L0 (K=16) and
    sigma/rgb (narrow M) blobs ship as separate small tensors.
  - vparts: the view-direction partial (Wv[:,32:] @ views + folded bias)
    is host-computed per point and injected into the view PSUM bank via
    an identity-stationary matmul (keeps the PE on uniform K=128 and
    dodges the has_written hazard).
  - DMA: only the two HWDGE queues (sync=SP, scalar=ACT) trigger DMAs;
    gpsimd SWDGE measured ~88GB/s with ~1us engine time per DMA - not
    used. Big tensors alternate between the queues; triggers are
    priority-ordered by pipeline window. Output is bf16 and streamed
    per window instead of one tail DMA.
  - Engine split: vector does the big PSUM->SBUF relu/copies (c_l0,
    c_l1, c_view), scalar does the narrow sigma/rgb copies + DMA
    triggers. gpsimd has no PSUM port and idles.
  - Biases: L0 bias rides in the matmul via a constant-1 input row;
    feat layer is folded into the view layer on the host. L1 bias is
    zero in practice (fast path: single relu copy); nonzero b1 falls
    back to per-quad tensor_scalar ops. sigma/rgb biases are added on
    host.
"""

import sys

import numpy as np
import ml_dtypes

BF16 = ml_dtypes.bfloat16

for _p in ("/opt/trn_rl_repo",):
    if _p not in sys.path:
        sys.path.append(_p)

NUM_MODELS = 4096
W = 32
N = 131072
NCORES = 8
EPG = 32               # experts per group per core (8 quads)
QPG = EPG // 4         # quads per group
NGROUPS = 512 // EPG
WIN = NCORES * EPG     # experts per capacity window

# wblob per group [128, WBLOB_F] bf16 (dense, host-built):
#   sigma lhsT [0:4Q)     rows 32b+h, col 4q+b
#   rgb lhsT   [4Q:16Q)   rows 32b+h, col 12q+3b+r
#   b1 bias    [16Q:17Q)  rows 32b+h, col q
WBLOB_F = 17 * QPG
# sblob per group [16, 128Q]: w0aug lhsT, rows 4b+k, col 128q+32b+h
SBLOB_F = 128 * QPG
# bdlv per group [128, 2*128*QPG]: L1 block-diag | viewA block-diag
BDG = 128 * QPG
BDLV_F = 2 * BDG
BANK = 512
PIPE = 4


def _prep(x, model_indices, pts_w0, pts_b0, pts_w1, pts_b1,
          feat_w, feat_b, sigma_w, sigma_b, view_w, view_b, rgb_w, rgb_b):
    """Host-side routing + packing. Returns per-core device arrays and
    decode info."""
    x = np.asarray(x, np.float32)
    idx = np.asarray(model_indices).astype(np.int64)
    counts = np.bincount(idx, minlength=NUM_MODELS)

    expert_order = np.argsort(-counts, kind="stable")  # descending count
    caps = np.empty(NGROUPS, np.int64)
    for k in range(NGROUPS):
        win = expert_order[WIN * k:WIN * (k + 1)]
        c = int(counts[win].max())
        caps[k] = max(4, -(-c // 4) * 4)  # round up to multiple of 4, >=4
    assert caps.max() * QPG <= BANK, "group capacity exceeds one PSUM bank"
    colstart = np.concatenate([[0], np.cumsum(QPG * caps)])
    w_tot = int(colstart[-1])

    order_pts = np.argsort(idx, kind="stable")
    starts = np.concatenate([[0], np.cumsum(counts)])

    # fold the feat layer into the view layer on the host:
    #   view(h) = relu(Wv [feat(h); views] + bv)
    #           = relu((Wv[:, :32] @ Wf) h + WvB views + (bv + Wv[:, :32] bf))
    vb_fold = view_b + np.einsum("goh,gh->go", view_w[:, :, :W], feat_b)
    vwA_fold = np.einsum("gox,gxh->goh", view_w[:, :, :W], feat_w)
    # host-computed view-direction partial per point:
    #   vpart = Wv[:, 32:] @ views + vb_fold   (injected into PSUM on device
    #   via an identity-stationary matmul, so both view matmuls are K=128)
    vw_g = view_w[idx][:, :, W:]                      # [N, 32, 3]
    vpart_all = (np.einsum("nij,nj->ni", vw_g, x[:, 3:6])
                 + vb_fold[idx]).astype(np.float32)   # [N, 32]
    w0aug = np.concatenate(
        [np.transpose(pts_w0, (0, 2, 1)), pts_b0[:, None, :]], axis=1
    ).astype(np.float32)                      # [E, 4, 32] lhsT rows: xyz+bias
    w1T = np.transpose(pts_w1, (0, 2, 1)).astype(np.float32)    # [E,32,32]
    vwAT = np.transpose(vwA_fold, (0, 2, 1)).astype(np.float32)
    sigT = np.transpose(sigma_w, (0, 2, 1)).astype(np.float32)  # [E,32,1]
    rgbT = np.transpose(rgb_w, (0, 2, 1)).astype(np.float32)    # [E,32,3]
    b1 = np.asarray(pts_b1, np.float32)

    per_core = []
    decode = []
    for c in range(NCORES):
        gq = np.stack([expert_order[WIN * k + EPG * c: WIN * k + EPG * (c + 1)]
                       for k in range(NGROUPS)])  # [NGROUPS, EPG]

        wblob = np.zeros((NGROUPS, 128, WBLOB_F), np.float32)
        sblob = np.zeros((NGROUPS, 16, SBLOB_F), np.float32)
        bdlv = np.zeros((NGROUPS, 128, BDLV_F), np.float32)
        xpts = np.zeros((16, w_tot), np.float32)
        vparts = np.zeros((128, w_tot), np.float32)
        xpts[3::4, :] = 1.0   # constant-1 rows for bias-in-matmul
        for k in range(NGROUPS):
            C = int(caps[k])
            col = int(colstart[k])
            for l in range(EPG):
                gid = int(gq[k, l])
                q, b = l // 4, l % 4
                wblob[k, 32 * b:32 * b + 32, 4 * q + b] = sigT[gid, :, 0]
                wblob[k, 32 * b:32 * b + 32, 4 * QPG + 12 * q + 3 * b:
                      4 * QPG + 12 * q + 3 * b + 3] = rgbT[gid]
                wblob[k, 32 * b:32 * b + 32, 16 * QPG + q] = b1[gid]
                sblob[k, 4 * b:4 * b + 4, 128 * q + 32 * b:
                      128 * q + 32 * b + 32] = w0aug[gid]
                bdlv[k, 32 * b:32 * b + 32,
                     128 * q + 32 * b:128 * q + 32 * b + 32] = w1T[gid]
                bdlv[k, 32 * b:32 * b + 32,
                     BDG + 128 * q + 32 * b:BDG + 128 * q + 32 * b + 32] = \
                    vwAT[gid]
                cnt = int(counts[gid])
                pts = order_pts[starts[gid]:starts[gid] + cnt]
                cq = col + q * C
                if cnt:
                    xpts[4 * b:4 * b + 3, cq:cq + cnt] = x[pts, :3].T
                    vparts[32 * b:32 * b + 32, cq:cq + cnt] = vpart_all[pts].T
                decode.append((c, gid, pts, q, b, cq, cnt))
        per_core.append(dict(
            xpts=xpts.astype(BF16), vparts=vparts.astype(BF16),
            ident=np.eye(128, dtype=np.float32).astype(BF16),
            wblob=wblob.transpose(1, 0, 2).reshape(128, NGROUPS * WBLOB_F)
                       .astype(BF16),
            sblob=sblob.transpose(1, 0, 2).reshape(16, NGROUPS * SBLOB_F)
                       .astype(BF16),
            bdlv=bdlv.transpose(1, 0, 2)
                     .reshape(128, NGROUPS * BDLV_F).astype(BF16)))

    b1_zero = not np.any(b1)
    return per_core, decode, caps, colstart, w_tot, b1_zero


def _build_nc(caps, w_tot, b1_zero):
    import concourse.mybir as mybir
    import concourse.tile as tile
    from concourse import bacc
    from contextlib import ExitStack

    f32 = mybir.dt.float32
    bf16 = mybir.dt.bfloat16
    ADD = mybir.AluOpType.add
    MAX = mybir.AluOpType.max

    nc = bacc.Bacc("TRN2", target_bir_lowering=False)
    xpts_d = nc.declare_dram_parameter("xpts", [16, w_tot], bf16, isOutput=False)
    vparts_d = nc.declare_dram_parameter("vparts", [128, w_tot], bf16,
                                         isOutput=False)
    ident_d = nc.declare_dram_parameter("ident", [128, 128], bf16,
                                        isOutput=False)
    wblob_d = nc.declare_dram_parameter("wblob", [128, NGROUPS * WBLOB_F], bf16,
                                        isOutput=False)
    sblob_d = nc.declare_dram_parameter("sblob", [16, NGROUPS * SBLOB_F], bf16,
                                        isOutput=False)
    bdlv_d = nc.declare_dram_parameter("bdlv", [128, NGROUPS * BDLV_F], bf16,
                                       isOutput=False)
    out_d = nc.declare_dram_parameter("out", [16, w_tot], bf16, isOutput=True)

    with tile.TileContext(nc) as tc, ExitStack() as ctx:
        const = ctx.enter_context(tc.tile_pool(name="const", bufs=1))
        hpool = ctx.enter_context(tc.tile_pool(name="h", bufs=4))
        pspool = ctx.enter_context(tc.tile_pool(name="ps", bufs=1, space="PSUM"))
        psall = pspool.tile([128, 8 * BANK], f32, tag="psall")
        # Global bank rotation; every step copies its slot out within the
        # step, so slot lifetime is one wave and 8 banks cover PIPE=4
        # windows without collisions.
        step_ctr = [0]

        def slot():
            bank = step_ctr[0] % 8
            step_ctr[0] += 1

            def mm_out(part_lo, m, q, C):
                base = bank * BANK + q * C
                return psall[part_lo:part_lo + m, base:base + C]

            def copy_src(part_lo, m, width):
                return psall[part_lo:part_lo + m,
                             bank * BANK:bank * BANK + width]

            return mm_out, copy_src

        xt = const.tile([16, w_tot], bf16)
        vpt = const.tile([128, w_tot], bf16)
        ident = const.tile([128, 128], bf16)
        wt_all = const.tile([128, NGROUPS * WBLOB_F], bf16)
        st_all = const.tile([16, NGROUPS * SBLOB_F], bf16)
        bdlv = const.tile([128, NGROUPS * BDLV_F], bf16)
        otr = const.tile([12, w_tot], bf16)
        ots = const.tile([4, w_tot], bf16)
        colstarts = np.concatenate([[0], np.cumsum(QPG * np.asarray(caps))])

        # ── PE warm-up ───────────────────────────────────────────────────
        # The HWDGE pipes take ~2.5us to deliver the first input, and the
        # PE HAM clock-gate needs ~3.4us of sustained activity to unlock
        # 2.4GHz. Fill the dead zone with dummy matmuls on a memset tile
        # so the real matmuls start warm.
        warm = const.tile([128, 256], bf16)
        nc.vector.memset(warm[:], 0.0)
        for _ in range(34):
            nc.tensor.matmul(out=psall[0:4, 7 * BANK:7 * BANK + 256],
                             lhsT=warm[:, 0:4], rhs=warm[:, 0:256],
                             start=True, stop=True, skip_group_check=True,
                             tile_position=(0, 0))

        # ── Priority-ordered input DMAs ──────────────────────────────────
        # Two HWDGE rings (sync=SP, scalar=ACT) carry ~5.1MB each at
        # ~160-180GB/s; gpsimd's slow SWDGE lane (~88GB/s, ~1us engine
        # time per DMA) carries mid-window small tensors + early outputs.
        # Entries are (priority, queue, dst, src); per-queue execution is
        # strictly in emission order, so order = need-time.
        S, A, G = 0, 1, 2

        def xs(a, b):
            return colstarts[a], colstarts[b]

        def bd(a, b, half=None):
            lo = a * BDLV_F + (BDG if half == "va" else 0)
            hi = b * BDLV_F - (BDG if half == "l1" else 0)
            return (bdlv[:, lo:hi], bdlv_d[:, lo:hi])

        def sl(t_sb, t_d, F, a, b):
            return (t_sb[:, a * F:b * F], t_d[:, a * F:b * F])

        dmas = []
        # window 0 critical path: L0(g0) needs sblob+xpts for g0 first.
        dmas += [(0, S, sl(st_all, sblob_d, SBLOB_F, 0, 1)),
                 (1, S, bd(0, 1, "l1")),
                 (2, S, bd(0, 1, "va")),
                 (3, S, sl(st_all, sblob_d, SBLOB_F, 1, 4)),
                 (4, S, bd(2, 3)),
                 (5, S, bd(3, 4)),
                 (6, S, bd(6, 8)),
                 (7, S, bd(10, 12)),
                 (8, S, bd(14, 16)),
                 (9, S, sl(st_all, sblob_d, SBLOB_F, 10, 16))]
        xlo0, xhi0 = xs(0, 1)
        xlo1, xhi1 = xs(1, 4)
        xlo4, xhi4 = xs(4, 8)
        xlo8, xhi8 = xs(8, 16)
        dmas += [(0, A, (xt[:, xlo0:xhi0], xpts_d[:, xlo0:xhi0])),
                 (1, A, sl(wt_all, wblob_d, WBLOB_F, 0, 1)),
                 (2, A, (vpt[:, xlo0:xhi0], vparts_d[:, xlo0:xhi0])),
                 (3, A, bd(1, 2, "l1")),
                 (4, A, bd(1, 2, "va")),
                 (5, A, (ident[:], ident_d[:])),
                 (6, A, (xt[:, xlo1:xhi1], xpts_d[:, xlo1:xhi1])),
                 (7, A, sl(wt_all, wblob_d, WBLOB_F, 1, 4)),
                 (8, A, (vpt[:, xlo1:xhi1], vparts_d[:, xlo1:xhi1])),
                 (9, A, bd(4, 6)),
                 (10, A, (vpt[:, xlo4:xhi4], vparts_d[:, xlo4:xhi4])),
                 (11, A, bd(8, 10)),
                 (12, A, (vpt[:, xlo8:xhi8], vparts_d[:, xlo8:xhi8])),
                 (13, A, bd(12, 14)),
                 (14, A, sl(wt_all, wblob_d, WBLOB_F, 10, 16))]
        xlo4f, xhi4f = xs(4, 16)
        dmas += [(0, G, (xt[:, xlo4f:xhi4f], xpts_d[:, xlo4f:xhi4f])),
                 (1, G, sl(st_all, sblob_d, SBLOB_F, 4, 10)),
                 (2, G, sl(wt_all, wblob_d, WBLOB_F, 4, 10))]
        engines = [nc.sync, nc.scalar, nc.gpsimd]
        byq = {}
        for (p, q, (dst, src)) in sorted(dmas, key=lambda t: (t[1], t[0])):
            engines[q].dma_start(out=dst, in_=src)
            byq[q] = byq.get(q, 0) + int(np.prod(dst.shape)) * 2
        assert abs(byq[S] - byq[A]) < 1.2e6, byq

        def group_steps(g):
            C = int(caps[g])
            WC = QPG * C
            col = int(colstarts[g])
            wt = wt_all[:, g * WBLOB_F:(g + 1) * WBLOB_F]
            st = st_all[:, g * SBLOB_F:(g + 1) * SBLOB_F]
            l1w = bdlv[:, g * BDLV_F:g * BDLV_F + BDG]
            vaw = bdlv[:, g * BDLV_F + BDG:(g + 1) * BDLV_F]
            state = {}

            def s_l0():
                mm0, cp0 = slot()
                for q in range(QPG):
                    nc.tensor.matmul(
                        out=mm0(0, 128, q, C),
                        lhsT=st[0:16, 128 * q:128 * q + 128],
                        rhs=xt[0:16, col + q * C:col + q * C + C],
                        start=True, stop=True, skip_group_check=True,
                        tile_position=(0, 0))
                state["cp0"] = cp0

            def c_l0():
                h1 = hpool.tile([128, WC], bf16, tag="h1")
                nc.vector.tensor_scalar_max(h1[:], state.pop("cp0")(0, 128, WC),
                                            0.0)
                state["h1"] = h1

            def s_l1():
                h1 = state.pop("h1")
                mm1, cp1 = slot()
                for q in range(QPG):
                    nc.tensor.matmul(
                        out=mm1(0, 128, q, C),
                        lhsT=l1w[:, 128 * q:128 * q + 128],
                        rhs=h1[:, q * C:q * C + C],
                        start=True, stop=True, skip_group_check=True,
                        tile_position=(0, 0))
                state["cp1"] = cp1

            def c_l1():
                cp1 = state.pop("cp1")
                h2 = hpool.tile([128, WC], bf16, tag="h2")
                if b1_zero:
                    nc.vector.tensor_scalar_max(h2[:], cp1(0, 128, WC), 0.0)
                else:
                    for q in range(QPG):
                        nc.vector.tensor_scalar(
                            out=h2[:, q * C:q * C + C],
                            in0=cp1(0, 128, WC)[:, q * C:q * C + C],
                            scalar1=wt[:, 16 * QPG + q:16 * QPG + q + 1],
                            scalar2=0.0, op0=ADD, op1=MAX)
                state["h2"] = h2

            def s_sigma():
                h2 = state["h2"]
                mms_, cps = slot()
                for q in range(QPG):
                    nc.tensor.matmul(
                        out=mms_(0, 4, q, C),
                        lhsT=wt[:, 4 * q:4 * q + 4],
                        rhs=h2[:, q * C:q * C + C],
                        start=True, stop=True, skip_group_check=True,
                        tile_position=(0, 0))
                state["cps"] = cps

            def c_sigma():
                nc.scalar.copy(ots[:, col:col + WC],
                               state.pop("cps")(0, 4, WC))

            def s_view():
                h2 = state.pop("h2")
                mmv, cpv = slot()
                # ident (start) and viewA (stop) stay adjacent per quad:
                # start=True clears has_written for the whole bank, so any
                # other start=True between an open pair loses the ident
                # contribution. Both are K=128 (uniform row size keeps the
                # PE pipeline from flushing between matmuls).
                for q in range(QPG):
                    nc.tensor.matmul(
                        out=mmv(0, 128, q, C),
                        lhsT=ident[:],
                        rhs=vpt[:, col + q * C:col + q * C + C],
                        start=True, stop=False, skip_group_check=True,
                        tile_position=(0, 0))
                    nc.tensor.matmul(
                        out=mmv(0, 128, q, C),
                        lhsT=vaw[:, 128 * q:128 * q + 128],
                        rhs=h2[:, q * C:q * C + C],
                        start=False, stop=True, skip_group_check=True,
                        tile_position=(0, 0))
                state["cpv"] = cpv

            def c_view():
                hv = hpool.tile([128, WC], bf16, tag="hv")
                nc.vector.tensor_scalar_max(hv[:], state.pop("cpv")(0, 128, WC),
                                            0.0)
                state["hv"] = hv

            def s_rgb():
                hv = state.pop("hv")
                mmr, cpr = slot()
                for q in range(QPG):
                    nc.tensor.matmul(
                        out=mmr(0, 12, q, C),
                        lhsT=wt[:, 4 * QPG + 12 * q:4 * QPG + 12 * q + 12],
                        rhs=hv[:, q * C:q * C + C],
                        start=True, stop=True, skip_group_check=True,
                        tile_position=(0, 0))
                state["cpr"] = cpr

            def c_rgb():
                nc.scalar.copy(otr[:, col:col + WC],
                               state.pop("cpr")(0, 12, WC))

            return [(s_l0, c_l0), (s_l1, c_l1), (s_sigma, c_sigma),
                    (s_view, c_view), (s_rgb, c_rgb)]

        for base in range(0, NGROUPS, PIPE):
            top = min(base + PIPE, NGROUPS)
            window = [group_steps(g) for g in range(base, top)]
            for stepi in range(5):
                for steps in window:
                    steps[stepi][0]()   # matmuls of the wave first
                for steps in window:
                    steps[stepi][1]()   # then the copies (waits satisfied)
            # stream this window's output while later windows compute;
            # early windows go out the idle SWDGE lane, the last window
            # uses the (by then empty) HWDGE rings.
            lo, hi = colstarts[base], colstarts[top]
            if top < NGROUPS:
                nc.gpsimd.dma_start(out=out_d[0:12, lo:hi], in_=otr[:, lo:hi])
                nc.gpsimd.dma_start(out=out_d[12:16, lo:hi], in_=ots[:, lo:hi])
            else:
                nc.sync.dma_start(out=out_d[0:12, lo:hi], in_=otr[:, lo:hi])
                nc.scalar.dma_start(out=out_d[12:16, lo:hi],
                                    in_=ots[:, lo:hi])

    nc.compile()
    return nc


def _decode_out(results, decode, sigma_b, rgb_b):
    y = np.empty((N, 4), np.float32)
    outs = [np.asarray(r["out"], dtype=np.float32) for r in results]
    for (c, gid, pts, q, b, cq, cnt) in decode:
        if cnt == 0:
            continue
        o = outs[c]
        y[pts, 0:3] = o[3 * b:3 * b + 3, cq:cq + cnt].T + rgb_b[gid]
        y[pts, 3] = o[12 + b, cq:cq + cnt] + sigma_b[gid, 0]
    return y


def kernel(**inputs):
    from concourse.bass_utils import run_bass_kernel_spmd

    per_core, decode, caps, colstart, w_tot, b1_zero = _prep(**inputs)
    nc = _build_nc(caps, w_tot, b1_zero)
    in_maps = [per_core[c] for c in range(NCORES)]
    res = run_bass_kernel_spmd(nc, in_maps, list(range(NCORES)))
    return _decode_out(res.results, decode,
                       np.asarray(inputs["sigma_b"], np.float32),
                       np.asarray(inputs["rgb_b"], np.float32))


# ---------------------------------------------------------------------------
# numpy emulation of the device program (for layout validation in test.py)
def _emulate_core(arrs, caps, w_tot):
    arrs = {k: np.asarray(v, np.float32) for k, v in arrs.items()}
    xt = arrs["xpts"]
    vpt = arrs["vparts"]
    bdlv = arrs["bdlv"]
    out = np.zeros((16, w_tot), np.float32)
    col = 0
    for g in range(NGROUPS):
        C = int(caps[g])
        WC = QPG * C
        wt = arrs["wblob"][:, g * WBLOB_F:(g + 1) * WBLOB_F]
        st = arrs["sblob"][:, g * SBLOB_F:(g + 1) * SBLOB_F]
        l1w = bdlv[:, g * BDLV_F:g * BDLV_F + BDG]
        vaw = bdlv[:, g * BDLV_F + BDG:(g + 1) * BDLV_F]

        h1 = np.zeros((128, WC), np.float32)
        for q in range(QPG):
            h1[:, q * C:q * C + C] = (
                st[:, 128 * q:128 * q + 128].T
                @ xt[:, col + q * C:col + q * C + C])
        h1 = np.maximum(h1, 0).astype(BF16).astype(np.float32)
        h2 = np.zeros((128, WC), np.float32)
        for q in range(QPG):
            h2[:, q * C:q * C + C] = (
                l1w[:, 128 * q:128 * q + 128].T @ h1[:, q * C:q * C + C]
                + wt[:, 16 * QPG + q:16 * QPG + q + 1])
        h2 = np.maximum(h2, 0).astype(BF16).astype(np.float32)
        for q in range(QPG):
            out[12:16, col + q * C:col + q * C + C] = (
                wt[:, 4 * q:4 * q + 4].T @ h2[:, q * C:q * C + C])
        hv = np.zeros((128, WC), np.float32)
        for q in range(QPG):
            hv[:, q * C:q * C + C] = (
                vaw[:, 128 * q:128 * q + 128].T @ h2[:, q * C:q * C + C]
                + vpt[:, col + q * C:col + q * C + C])
        hv = np.maximum(hv, 0).astype(BF16).astype(np.float32)
        for q in range(QPG):
            out[0:12, col + q * C:col + q * C + C] = (
                wt[:, 4 * QPG + 12 * q:4 * QPG + 12 * q + 12].T
                @ hv[:, q * C:q * C + C])
        col += WC
    return out.astype(BF16)


def kernel_emulated(**inputs):
    per_core, decode, caps, colstart, w_tot, b1_zero = _prep(**inputs)
    results = [{"out": _emulate_core(per_core[c], caps, w_tot)}
               for c in range(NCORES)]
    return _decode_out(results, decode,
                       np.asarray(inputs["sigma_b"], np.float32),
                       np.asarray(inputs["rgb_b"], np.float32))


# revision 5
# speedup vs baseline: 1.0225x; 1.0087x over previous
"""BatchedKiloNeRF Trainium2 kernel.

Strategy (expert-parallel, host routing, bf16, block-diagonal quads):
  - 4096 tiny MLPs ("experts"), 131072 points routed by model_indices.
  - PE cost is dominated by the 128-col LDWEIGHTS (~53ns with FWL), so
    experts are packed 4-per-matmul: quad q stacks experts b=0..3 on
    partition bands 32b with a block-diagonal stationary [128, 128];
    one matmul streams C point-columns for 4 experts at once.
  - Host sorts experts by point count and packs groups of EPG=32 (8 quads)
    per core; points padded to the group capacity C (max count in the
    8*EPG-expert window). Hidden states are [128, 8C] bf16 SBUF tiles:
    partition band 32b = expert band, C-column segment q = quad.
  - PSUM: each step claims one full bank; 8-bank global rotation. All
    matmuls of a step are closed single-shots except the view step:
    ident (start) and viewA (stop) are emitted adjacently per quad
    because start=True clears has_written for the WHOLE bank - any
    interleaving of other start=True matmuls between an open pair
    loses the first contribution.
  - Weights ship dense from host: bdlv packs the L1 and viewA block-
    diagonal stationaries column-adjacent per group ([128, 2048]/group)
    so chunked DMAs have 16KB per-partition runs. L0 (K=16) and
    sigma/rgb (narrow M) blobs ship as separate small tensors.
  - vparts: the view-direction partial (Wv[:,32:] @ views + folded bias)
    is host-computed per point and injected into the view PSUM bank via
    an identity-stationary matmul (keeps the PE on uniform K=128 and
    dodges the has_written hazard).
  - DMA: only the two HWDGE queues (sync=SP, scalar=ACT) trigger DMAs;
    gpsimd SWDGE measured ~88GB/s with ~1us engine time per DMA - not
    used. Big tensors alternate between the queues; triggers are
    priority-ordered by pipeline window. Output is bf16 and streamed
    per window instead of one tail DMA.
  - Engine split: vector does the big PSUM->SBUF relu/copies (c_l0,
    c_l1, c_view), scalar does the narrow sigma/rgb copies + DMA
    triggers. gpsimd has no PSUM port and idles.
  - Biases: L0 bias rides in the matmul via a constant-1 input row;
    feat layer is folded into the view layer on the host. L1 bias is
    zero in practice (fast path: single relu copy); nonzero b1 falls
    back to per-quad tensor_scalar ops. sigma/rgb biases are added on
    host.
"""

import sys

import numpy as np
import ml_dtypes

BF16 = ml_dtypes.bfloat16

for _p in ("/opt/trn_rl_repo",):
    if _p not in sys.path:
        sys.path.append(_p)

NUM_MODELS = 4096
W = 32
N = 131072
NCORES = 8
EPG = 32               # experts per group per core (8 quads)
QPG = EPG // 4         # quads per group
NGROUPS = 512 // EPG
WIN = NCORES * EPG     # experts per capacity window

# wblob per group [128, WBLOB_F] bf16 (dense, host-built):
#   sigma lhsT [0:4Q)     rows 32b+h, col 4q+b
#   rgb lhsT   [4Q:16Q)   rows 32b+h, col 12q+3b+r
#   b1 bias    [16Q:17Q)  rows 32b+h, col q
WBLOB_F = 17 * QPG
# sblob per group [16, 128Q]: w0aug lhsT, rows 4b+k, col 128q+32b+h
SBLOB_F = 128 * QPG
# bdlv per group [128, 2*128*QPG]: L1 block-diag | viewA block-diag
BDG = 128 * QPG
BDLV_F = 2 * BDG
BANK = 512
PIPE = 4


def _prep(x, model_indices, pts_w0, pts_b0, pts_w1, pts_b1,
          feat_w, feat_b, sigma_w, sigma_b, view_w, view_b, rgb_w, rgb_b):
    """Host-side routing + packing. Returns per-core device arrays and
    decode info."""
    x = np.asarray(x, np.float32)
    idx = np.asarray(model_indices).astype(np.int64)
    counts = np.bincount(idx, minlength=NUM_MODELS)

    expert_order = np.argsort(-counts, kind="stable")  # descending count
    caps = np.empty(NGROUPS, np.int64)
    for k in range(NGROUPS):
        win = expert_order[WIN * k:WIN * (k + 1)]
        c = int(counts[win].max())
        caps[k] = max(4, -(-c // 4) * 4)  # round up to multiple of 4, >=4
    assert caps.max() * QPG <= BANK, "group capacity exceeds one PSUM bank"
    colstart = np.concatenate([[0], np.cumsum(QPG * caps)])
    w_tot = int(colstart[-1])

    order_pts = np.argsort(idx, kind="stable")
    starts = np.concatenate([[0], np.cumsum(counts)])

    # fold the feat layer into the view layer on the host:
    #   view(h) = relu(Wv [feat(h); views] + bv)
    #           = relu((Wv[:, :32] @ Wf) h + WvB views + (bv + Wv[:, :32] bf))
    vb_fold = view_b + np.einsum("goh,gh->go", view_w[:, :, :W], feat_b)
    vwA_fold = np.einsum("gox,gxh->goh", view_w[:, :, :W], feat_w)
    # host-computed view-direction partial per point:
    #   vpart = Wv[:, 32:] @ views + vb_fold   (injected into PSUM on device
    #   via an identity-stationary matmul, so both view matmuls are K=128)
    vw_g = view_w[idx][:, :, W:]                      # [N, 32, 3]
    vpart_all = (np.einsum("nij,nj->ni", vw_g, x[:, 3:6])
                 + vb_fold[idx]).astype(np.float32)   # [N, 32]
    w0aug = np.concatenate(
        [np.transpose(pts_w0, (0, 2, 1)), pts_b0[:, None, :]], axis=1
    ).astype(np.float32)                      # [E, 4, 32] lhsT rows: xyz+bias
    w1T = np.transpose(pts_w1, (0, 2, 1)).astype(np.float32)    # [E,32,32]
    vwAT = np.transpose(vwA_fold, (0, 2, 1)).astype(np.float32)
    sigT = np.transpose(sigma_w, (0, 2, 1)).astype(np.float32)  # [E,32,1]
    rgbT = np.transpose(rgb_w, (0, 2, 1)).astype(np.float32)    # [E,32,3]
    b1 = np.asarray(pts_b1, np.float32)

    per_core = []
    decode = []
    for c in range(NCORES):
        gq = np.stack([expert_order[WIN * k + EPG * c: WIN * k + EPG * (c + 1)]
                       for k in range(NGROUPS)])  # [NGROUPS, EPG]

        wblob = np.zeros((NGROUPS, 128, WBLOB_F), np.float32)
        sblob = np.zeros((NGROUPS, 16, SBLOB_F), np.float32)
        bdlv = np.zeros((NGROUPS, 128, BDLV_F), np.float32)
        xpts = np.zeros((16, w_tot), np.float32)
        vparts = np.zeros((128, w_tot), np.float32)
        xpts[3::4, :] = 1.0   # constant-1 rows for bias-in-matmul
        for k in range(NGROUPS):
            C = int(caps[k])
            col = int(colstart[k])
            for l in range(EPG):
                gid = int(gq[k, l])
                q, b = l // 4, l % 4
                wblob[k, 32 * b:32 * b + 32, 4 * q + b] = sigT[gid, :, 0]
                wblob[k, 32 * b:32 * b + 32, 4 * QPG + 12 * q + 3 * b:
                      4 * QPG + 12 * q + 3 * b + 3] = rgbT[gid]
                wblob[k, 32 * b:32 * b + 32, 16 * QPG + q] = b1[gid]
                sblob[k, 4 * b:4 * b + 4, 128 * q + 32 * b:
                      128 * q + 32 * b + 32] = w0aug[gid]
                bdlv[k, 32 * b:32 * b + 32,
                     128 * q + 32 * b:128 * q + 32 * b + 32] = w1T[gid]
                bdlv[k, 32 * b:32 * b + 32,
                     BDG + 128 * q + 32 * b:BDG + 128 * q + 32 * b + 32] = \
                    vwAT[gid]
                cnt = int(counts[gid])
                pts = order_pts[starts[gid]:starts[gid] + cnt]
                cq = col + q * C
                if cnt:
                    xpts[4 * b:4 * b + 3, cq:cq + cnt] = x[pts, :3].T
                    vparts[32 * b:32 * b + 32, cq:cq + cnt] = vpart_all[pts].T
                decode.append((c, gid, pts, q, b, cq, cnt))
        per_core.append(dict(
            xpts=xpts.astype(BF16), vparts=vparts.astype(BF16),
            ident=np.eye(128, dtype=np.float32).astype(BF16),
            wblob=wblob.transpose(1, 0, 2).reshape(128, NGROUPS * WBLOB_F)
                       .astype(BF16),
            sblob=sblob.transpose(1, 0, 2).reshape(16, NGROUPS * SBLOB_F)
                       .astype(BF16),
            bdlv=bdlv.transpose(1, 0, 2)
                     .reshape(128, NGROUPS * BDLV_F).astype(BF16)))

    b1_zero = not np.any(b1)
    return per_core, decode, caps, colstart, w_tot, b1_zero


def _build_nc(caps, w_tot, b1_zero):
    import concourse.mybir as mybir
    import concourse.tile as tile
    from concourse import bacc
    from contextlib import ExitStack

    f32 = mybir.dt.float32
    bf16 = mybir.dt.bfloat16
    ADD = mybir.AluOpType.add
    MAX = mybir.AluOpType.max

    nc = bacc.Bacc("TRN2", target_bir_lowering=False)
    xpts_d = nc.declare_dram_parameter("xpts", [16, w_tot], bf16, isOutput=False)
    vparts_d = nc.declare_dram_parameter("vparts", [128, w_tot], bf16,
                                         isOutput=False)
    ident_d = nc.declare_dram_parameter("ident", [128, 128], bf16,
                                        isOutput=False)
    wblob_d = nc.declare_dram_parameter("wblob", [128, NGROUPS * WBLOB_F], bf16,
                                        isOutput=False)
    sblob_d = nc.declare_dram_parameter("sblob", [16, NGROUPS * SBLOB_F], bf16,
                                        isOutput=False)
    bdlv_d = nc.declare_dram_parameter("bdlv", [128, NGROUPS * BDLV_F], bf16,
                                       isOutput=False)
    out_d = nc.declare_dram_parameter("out", [16, w_tot], bf16, isOutput=True)

    with tile.TileContext(nc) as tc, ExitStack() as ctx:
        const = ctx.enter_context(tc.tile_pool(name="const", bufs=1))
        hpool = ctx.enter_context(tc.tile_pool(name="h", bufs=4))
        pspool = ctx.enter_context(tc.tile_pool(name="ps", bufs=1, space="PSUM"))
        psall = pspool.tile([128, 8 * BANK], f32, tag="psall")
        # Global bank rotation; every step copies its slot out within the
        # step, so slot lifetime is one wave and 8 banks cover PIPE=4
        # windows without collisions.
        step_ctr = [0]

        def slot():
            bank = step_ctr[0] % 8
            step_ctr[0] += 1

            def mm_out(part_lo, m, q, C):
                base = bank * BANK + q * C
                return psall[part_lo:part_lo + m, base:base + C]

            def copy_src(part_lo, m, width):
                return psall[part_lo:part_lo + m,
                             bank * BANK:bank * BANK + width]

            return mm_out, copy_src

        xt = const.tile([16, w_tot], bf16)
        vpt = const.tile([128, w_tot], bf16)
        ident = const.tile([128, 128], bf16)
        wt_all = const.tile([128, NGROUPS * WBLOB_F], bf16)
        st_all = const.tile([16, NGROUPS * SBLOB_F], bf16)
        bdlv = const.tile([128, NGROUPS * BDLV_F], bf16)
        otr = const.tile([12, w_tot], bf16)
        ots = const.tile([4, w_tot], bf16)
        colstarts = np.concatenate([[0], np.cumsum(QPG * np.asarray(caps))])

        # ── PE warm-up ───────────────────────────────────────────────────
        # The HWDGE pipes take ~2.5us to deliver the first input, and the
        # PE HAM clock-gate needs ~3.4us of sustained activity to unlock
        # 2.4GHz. Fill the dead zone with dummy matmuls on a memset tile
        # so the real matmuls start warm.
        warm = const.tile([128, 16], bf16)
        nc.vector.memset(warm[:], 0.0)
        for _ in range(56):
            nc.tensor.matmul(out=psall[0:4, 7 * BANK:7 * BANK + 16],
                             lhsT=warm[:, 0:4], rhs=warm[:, 0:16],
                             start=True, stop=True, skip_group_check=True,
                             tile_position=(0, 0))

        # ── Priority-ordered input DMAs ──────────────────────────────────
        # Two HWDGE rings (sync=SP, scalar=ACT) carry ~5.1MB each at
        # ~160-180GB/s; gpsimd's slow SWDGE lane (~88GB/s, ~1us engine
        # time per DMA) carries mid-window small tensors + early outputs.
        # Entries are (priority, queue, dst, src); per-queue execution is
        # strictly in emission order, so order = need-time.
        S, A, G = 0, 1, 2

        def xs(a, b):
            return colstarts[a], colstarts[b]

        def bd(a, b, half=None):
            lo = a * BDLV_F + (BDG if half == "va" else 0)
            hi = b * BDLV_F - (BDG if half == "l1" else 0)
            return (bdlv[:, lo:hi], bdlv_d[:, lo:hi])

        def sl(t_sb, t_d, F, a, b):
            return (t_sb[:, a * F:b * F], t_d[:, a * F:b * F])

        dmas = []
        # window 0 critical path: L0(g0) needs sblob+xpts for g0 first.
        dmas += [(0, S, sl(st_all, sblob_d, SBLOB_F, 0, 1)),
                 (1, S, bd(0, 1, "l1")),
                 (2, S, bd(0, 1, "va")),
                 (3, S, sl(st_all, sblob_d, SBLOB_F, 1, 4)),
                 (4, S, bd(2, 3)),
                 (5, S, bd(3, 4)),
                 (6, S, bd(4, 6)),
                 (7, S, bd(8, 10)),
                 (8, S, bd(12, 14))]
        xlo0, xhi0 = xs(0, 1)
        xlo1, xhi1 = xs(1, 4)
        xlo4, xhi4 = xs(4, 8)
        xlo8, xhi8 = xs(8, 16)
        dmas += [(0, A, (xt[:, xlo0:xhi0], xpts_d[:, xlo0:xhi0])),
                 (1, A, sl(wt_all, wblob_d, WBLOB_F, 0, 1)),
                 (2, A, (vpt[:, xlo0:xhi0], vparts_d[:, xlo0:xhi0])),
                 (3, A, bd(1, 2, "l1")),
                 (4, A, bd(1, 2, "va")),
                 (5, A, (ident[:], ident_d[:])),
                 (6, A, (xt[:, xlo1:xhi1], xpts_d[:, xlo1:xhi1])),
                 (7, A, sl(wt_all, wblob_d, WBLOB_F, 1, 4)),
                 (8, A, (vpt[:, xlo1:xhi1], vparts_d[:, xlo1:xhi1])),
                 (9, A, bd(6, 8)),
                 (10, A, (vpt[:, xlo4:xhi4], vparts_d[:, xlo4:xhi4])),
                 (11, A, bd(10, 12)),
                 (12, A, (vpt[:, xlo8:xhi8], vparts_d[:, xlo8:xhi8])),
                 (13, A, bd(14, 16)),
                 (14, A, sl(st_all, sblob_d, SBLOB_F, 10, 16)),
                 (15, A, sl(wt_all, wblob_d, WBLOB_F, 10, 16))]
        xlo4f, xhi4f = xs(4, 16)
        dmas += [(0, G, (xt[:, xlo4f:xhi4f], xpts_d[:, xlo4f:xhi4f])),
                 (1, G, sl(st_all, sblob_d, SBLOB_F, 4, 10)),
                 (2, G, sl(wt_all, wblob_d, WBLOB_F, 4, 10))]
        engines = [nc.sync, nc.scalar, nc.gpsimd]
        byq = {}
        for (p, q, (dst, src)) in sorted(dmas, key=lambda t: (t[1], t[0])):
            engines[q].dma_start(out=dst, in_=src)
            byq[q] = byq.get(q, 0) + int(np.prod(dst.shape)) * 2
        assert abs(byq[S] - byq[A]) < 1.2e6, byq

        def group_steps(g):
            C = int(caps[g])
            WC = QPG * C
            col = int(colstarts[g])
            wt = wt_all[:, g * WBLOB_F:(g + 1) * WBLOB_F]
            st = st_all[:, g * SBLOB_F:(g + 1) * SBLOB_F]
            l1w = bdlv[:, g * BDLV_F:g * BDLV_F + BDG]
            vaw = bdlv[:, g * BDLV_F + BDG:(g + 1) * BDLV_F]
            state = {}

            def s_l0():
                mm0, cp0 = slot()
                for q in range(QPG):
                    nc.tensor.matmul(
                        out=mm0(0, 128, q, C),
                        lhsT=st[0:16, 128 * q:128 * q + 128],
                        rhs=xt[0:16, col + q * C:col + q * C + C],
                        start=True, stop=True, skip_group_check=True,
                        tile_position=(0, 0))
                state["cp0"] = cp0

            def c_l0():
                h1 = hpool.tile([128, WC], bf16, tag="h1")
                nc.vector.tensor_scalar_max(h1[:], state.pop("cp0")(0, 128, WC),
                                            0.0)
                state["h1"] = h1

            def s_l1():
                h1 = state.pop("h1")
                mm1, cp1 = slot()
                for q in range(QPG):
                    nc.tensor.matmul(
                        out=mm1(0, 128, q, C),
                        lhsT=l1w[:, 128 * q:128 * q + 128],
                        rhs=h1[:, q * C:q * C + C],
                        start=True, stop=True, skip_group_check=True,
                        tile_position=(0, 0))
                state["cp1"] = cp1

            def c_l1():
                cp1 = state.pop("cp1")
                h2 = hpool.tile([128, WC], bf16, tag="h2")
                if b1_zero:
                    nc.vector.tensor_scalar_max(h2[:], cp1(0, 128, WC), 0.0)
                else:
                    for q in range(QPG):
                        nc.vector.tensor_scalar(
                            out=h2[:, q * C:q * C + C],
                            in0=cp1(0, 128, WC)[:, q * C:q * C + C],
                            scalar1=wt[:, 16 * QPG + q:16 * QPG + q + 1],
                            scalar2=0.0, op0=ADD, op1=MAX)
                state["h2"] = h2

            def s_sigma():
                h2 = state["h2"]
                mms_, cps = slot()
                for q in range(QPG):
                    nc.tensor.matmul(
                        out=mms_(0, 4, q, C),
                        lhsT=wt[:, 4 * q:4 * q + 4],
                        rhs=h2[:, q * C:q * C + C],
                        start=True, stop=True, skip_group_check=True,
                        tile_position=(0, 0))
                state["cps"] = cps

            def c_sigma():
                nc.scalar.copy(ots[:, col:col + WC],
                               state.pop("cps")(0, 4, WC))

            def s_view():
                h2 = state.pop("h2")
                mmv, cpv = slot()
                # ident (start) and viewA (stop) stay adjacent per quad:
                # start=True clears has_written for the whole bank, so any
                # other start=True between an open pair loses the ident
                # contribution. Both are K=128 (uniform row size keeps the
                # PE pipeline from flushing between matmuls).
                for q in range(QPG):
                    nc.tensor.matmul(
                        out=mmv(0, 128, q, C),
                        lhsT=ident[:],
                        rhs=vpt[:, col + q * C:col + q * C + C],
                        start=True, stop=False, skip_group_check=True,
                        tile_position=(0, 0))
                    nc.tensor.matmul(
                        out=mmv(0, 128, q, C),
                        lhsT=vaw[:, 128 * q:128 * q + 128],
                        rhs=h2[:, q * C:q * C + C],
                        start=False, stop=True, skip_group_check=True,
                        tile_position=(0, 0))
                state["cpv"] = cpv

            def c_view():
                hv = hpool.tile([128, WC], bf16, tag="hv")
                nc.vector.tensor_scalar_max(hv[:], state.pop("cpv")(0, 128, WC),
                                            0.0)
                state["hv"] = hv

            def s_rgb():
                hv = state.pop("hv")
                mmr, cpr = slot()
                for q in range(QPG):
                    nc.tensor.matmul(
                        out=mmr(0, 12, q, C),
                        lhsT=wt[:, 4 * QPG + 12 * q:4 * QPG + 12 * q + 12],
                        rhs=hv[:, q * C:q * C + C],
                        start=True, stop=True, skip_group_check=True,
                        tile_position=(0, 0))
                state["cpr"] = cpr

            def c_rgb():
                nc.scalar.copy(otr[:, col:col + WC],
                               state.pop("cpr")(0, 12, WC))

            return [(s_l0, c_l0), (s_l1, c_l1), (s_sigma, c_sigma),
                    (s_view, c_view), (s_rgb, c_rgb)]

        for base in range(0, NGROUPS, PIPE):
            top = min(base + PIPE, NGROUPS)
            window = [group_steps(g) for g in range(base, top)]
            for stepi in range(5):
                for steps in window:
                    steps[stepi][0]()   # matmuls of the wave first
                for steps in window:
                    steps[stepi][1]()   # then the copies (waits satisfied)
            # stream this window's output while later windows compute;
            # early windows go out the idle SWDGE lane, the last window
            # uses the (by then empty) HWDGE rings.
            lo, hi = colstarts[base], colstarts[top]
            if top < NGROUPS:
                nc.gpsimd.dma_start(out=out_d[0:12, lo:hi], in_=otr[:, lo:hi])
                nc.gpsimd.dma_start(out=out_d[12:16, lo:hi], in_=ots[:, lo:hi])
            else:
                nc.sync.dma_start(out=out_d[0:12, lo:hi], in_=otr[:, lo:hi])
                nc.scalar.dma_start(out=out_d[12:16, lo:hi],
                                    in_=ots[:, lo:hi])

    nc.compile()
    return nc


def _decode_out(results, decode, sigma_b, rgb_b):
    y = np.empty((N, 4), np.float32)
    outs = [np.asarray(r["out"], dtype=np.float32) for r in results]
    for (c, gid, pts, q, b, cq, cnt) in decode:
        if cnt == 0:
            continue
        o = outs[c]
        y[pts, 0:3] = o[3 * b:3 * b + 3, cq:cq + cnt].T + rgb_b[gid]
        y[pts, 3] = o[12 + b, cq:cq + cnt] + sigma_b[gid, 0]
    return y


def kernel(**inputs):
    from concourse.bass_utils import run_bass_kernel_spmd

    per_core, decode, caps, colstart, w_tot, b1_zero = _prep(**inputs)
    nc = _build_nc(caps, w_tot, b1_zero)
    in_maps = [per_core[c] for c in range(NCORES)]
    res = run_bass_kernel_spmd(nc, in_maps, list(range(NCORES)))
    return _decode_out(res.results, decode,
                       np.asarray(inputs["sigma_b"], np.float32),
                       np.asarray(inputs["rgb_b"], np.float32))


# ---------------------------------------------------------------------------
# numpy emulation of the device program (for layout validation in test.py)
def _emulate_core(arrs, caps, w_tot):
    arrs = {k: np.asarray(v, np.float32) for k, v in arrs.items()}
    xt = arrs["xpts"]
    vpt = arrs["vparts"]
    bdlv = arrs["bdlv"]
    out = np.zeros((16, w_tot), np.float32)
    col = 0
    for g in range(NGROUPS):
        C = int(caps[g])
        WC = QPG * C
        wt = arrs["wblob"][:, g * WBLOB_F:(g + 1) * WBLOB_F]
        st = arrs["sblob"][:, g * SBLOB_F:(g + 1) * SBLOB_F]
        l1w = bdlv[:, g * BDLV_F:g * BDLV_F + BDG]
        vaw = bdlv[:, g * BDLV_F + BDG:(g + 1) * BDLV_F]

        h1 = np.zeros((128, WC), np.float32)
        for q in range(QPG):
            h1[:, q * C:q * C + C] = (
                st[:, 128 * q:128 * q + 128].T
                @ xt[:, col + q * C:col + q * C + C])
        h1 = np.maximum(h1, 0).astype(BF16).astype(np.float32)
        h2 = np.zeros((128, WC), np.float32)
        for q in range(QPG):
            h2[:, q * C:q * C + C] = (
                l1w[:, 128 * q:128 * q + 128].T @ h1[:, q * C:q * C + C]
                + wt[:, 16 * QPG + q:16 * QPG + q + 1])
        h2 = np.maximum(h2, 0).astype(BF16).astype(np.float32)
        for q in range(QPG):
            out[12:16, col + q * C:col + q * C + C] = (
                wt[:, 4 * q:4 * q + 4].T @ h2[:, q * C:q * C + C])
        hv = np.zeros((128, WC), np.float32)
        for q in range(QPG):
            hv[:, q * C:q * C + C] = (
                vaw[:, 128 * q:128 * q + 128].T @ h2[:, q * C:q * C + C]
                + vpt[:, col + q * C:col + q * C + C])
        hv = np.maximum(hv, 0).astype(BF16).astype(np.float32)
        for q in range(QPG):
            out[0:12, col + q * C:col + q * C + C] = (
                wt[:, 4 * QPG + 12 * q:4 * QPG + 12 * q + 12].T
                @ hv[:, q * C:q * C + C])
        col += WC
    return out.astype(BF16)


def kernel_emulated(**inputs):
    per_core, decode, caps, colstart, w_tot, b1_zero = _prep(**inputs)
    results = [{"out": _emulate_core(per_core[c], caps, w_tot)}
               for c in range(NCORES)]
    return _decode_out(results, decode,
                       np.asarray(inputs["sigma_b"], np.float32),
                       np.asarray(inputs["rgb_b"], np.float32))
